# revision 12
# baseline (speedup 1.0000x reference)
"""AGCB (patch non-local attention + 3x3 conv + BN + residual ReLU) on 8 TRN2 cores.

Data-parallel: batch 16 -> 2 examples/core. Per core, per example:
  - split 64x64 into 4 quadrants of 32x32 (L=1024), block-major x layout
  - q/k (d=8) projections; e'[j,i] = k^T q; exp on ACT (no max-sub; |e|<17)
  - denominator: ones-lhsT matmul over expE -> denom broadcast across all 128
    PSUM partitions; ones are scaled 1/gamma_nl so reciprocal gives
    gamma_nl/denom directly
  - v^T[l,c] = xb^T wvT; av[c,i] = sum_j vT[j,c] expE[j,i] (lhsT = vT slices)
  - ctx = av * rden + gamma_nl*bv + x  (2 DVE ops, written into padded image)
  - conv3x3 as 9 shifted matmuls over the zero-padded 66x66 image; BN/gamma/
    conv-bias folded on host into scale As[c] and x_pre = x + Bs[c]; relu.

All matmuls bf16 (f32 PSUM accumulation); host pre-converts weights/x to bf16.
"""
import os
import sys

import numpy as np
import ml_dtypes

for _p in ("/opt/trn_rl_repo", "/root/.axon_site/_ro/trn_rl_repo"):
    if os.path.isdir(_p) and _p not in sys.path:
        sys.path.insert(0, _p)

import concourse.bass as bass
import concourse.tile as tile
from concourse import mybir, bacc
from concourse.bass_utils import run_bass_kernel_spmd

BF16 = mybir.dt.bfloat16
F32 = mybir.dt.float32
N_CORES = 8
B, C, H, W = 16, 256, 64, 64
BL = B // N_CORES          # examples per core
S = 2                      # split factor
HB = H // S                # 32
L = HB * HB                # 1024
D = 8                      # q/k dim
CC = C // 128              # channel chunks (2)
BN_EPS = 1e-5

_cache = {}


def build_bass():
    """Build the per-core Bass program (same SPMD program on all 8 cores)."""
    nc = bacc.Bacc(None, target_bir_lowering=False)

    # ---- DRAM parameters (per-core shapes) ----
    x16_h = nc.declare_dram_parameter("x16", [BL, C, H * W], BF16, isOutput=False)
    xpre_h = nc.declare_dram_parameter("xpre", [BL, C, H * W], F32, isOutput=False)
    wq_h = nc.declare_dram_parameter("wqT", [CC, 128, D], BF16, isOutput=False)
    wk_h = nc.declare_dram_parameter("wkT", [CC, 128, D], BF16, isOutput=False)
    bq_h = nc.declare_dram_parameter("bq", [D, 1], F32, isOutput=False)
    bk_h = nc.declare_dram_parameter("bk", [D, 1], F32, isOutput=False)
    wv_h = nc.declare_dram_parameter("wvT", [CC, 128, C], BF16, isOutput=False)
    ones_h = nc.declare_dram_parameter("onesg", [128, 128], BF16, isOutput=False)
    gbv_h = nc.declare_dram_parameter("gnlbv", [CC, 128, 1], F32, isOutput=False)
    cw_h = nc.declare_dram_parameter("cw", [CC, 128, 9, C], BF16, isOutput=False)
    as_h = nc.declare_dram_parameter("As", [CC, 128, 1], F32, isOutput=False)
    out_h = nc.declare_dram_parameter("out", [BL, C, H * W], F32, isOutput=True)

    AF = mybir.ActivationFunctionType
    ALU = mybir.AluOpType

    with tile.TileContext(nc) as tc:
        with (
            tc.tile_pool(name="wpool", bufs=1) as wpool,
            tc.tile_pool(name="xpool", bufs=2) as xpool,
            tc.tile_pool(name="blk", bufs=3) as blk,
            tc.tile_pool(name="ctxp", bufs=2) as ctxp,
            tc.tile_pool(name="cvp", bufs=3) as cvp,
            tc.tile_pool(name="pbig", bufs=3, space="PSUM") as pbig,
            tc.tile_pool(name="pmisc", bufs=2, space="PSUM") as pmisc,
        ):
            # ---- HAM warmup: dense dummy matmuls on a memset tile ----
            warm_sb = wpool.tile([128, 512], BF16, name="warm_sb")
            nc.vector.memset(warm_sb, 0.25)
            pw = pmisc.tile([128, 512], F32, tag="pmisc", name="pmisc")
            for w in range(48):
                nc.tensor.matmul(
                    pw, warm_sb[:, 0:128], warm_sb, start=(w == 0), stop=(w == 47)
                )

            # ---- weights -> SBUF (once) ----
            wq_sb = wpool.tile([128, CC, D], BF16, name="wq_sb")
            wk_sb = wpool.tile([128, CC, D], BF16, name="wk_sb")
            wv_sb = wpool.tile([128, CC, C], BF16, name="wv_sb")
            cw_sb = wpool.tile([128, CC, 9, C], BF16, name="cw_sb")
            ones_sb = wpool.tile([128, 128], BF16, name="ones_sb")
            bq_sb = wpool.tile([D, 1], F32, name="bq_sb")
            bk_sb = wpool.tile([D, 1], F32, name="bk_sb")
            gbv_sb = wpool.tile([128, CC, 1], F32, name="gbv_sb")
            as_sb = wpool.tile([128, CC, 1], F32, name="as_sb")
            for cc in range(CC):
                nc.sync.dma_start(out=wq_sb[:, cc, :], in_=wq_h.ap()[cc])
                nc.sync.dma_start(out=wk_sb[:, cc, :], in_=wk_h.ap()[cc])
                nc.sync.dma_start(out=wv_sb[:, cc, :], in_=wv_h.ap()[cc])
                nc.sync.dma_start(out=cw_sb[:, cc, :, :], in_=cw_h.ap()[cc])
                nc.sync.dma_start(out=gbv_sb[:, cc, :], in_=gbv_h.ap()[cc])
                nc.sync.dma_start(out=as_sb[:, cc, :], in_=as_h.ap()[cc])
            nc.sync.dma_start(out=ones_sb, in_=ones_h.ap())
            nc.sync.dma_start(out=bq_sb, in_=bq_h.ap())
            nc.sync.dma_start(out=bk_sb, in_=bk_h.ap())

            for ex in range(BL):
                # ---- x (bf16, block-major) -> SBUF [128, 4, 1024] per chunk ----
                xb_sb = [
                    xpool.tile([128, S * S, L], BF16, tag=f"xb{cc}", name=f"xb{cc}")
                    for cc in range(CC)
                ]
                for cc in range(CC):
                    nc.sync.dma_start(
                        out=xb_sb[cc][:, :, :],
                        in_=x16_h.ap()[ex, cc * 128 : (cc + 1) * 128, :].rearrange(
                            "p (b l) -> p b l", b=S * S
                        ),
                    )
                # padded conv input images (zero border), one per channel chunk
                ctx_t = [
                    ctxp.tile([128, H + 2, W + 2], BF16, tag=f"ctx{cc}", name=f"ctx{cc}")
                    for cc in range(CC)
                ]
                for cc in range(CC):
                    nc.gpsimd.memset(ctx_t[cc][:, 0, :], 0.0)
                    nc.gpsimd.memset(ctx_t[cc][:, H + 1, :], 0.0)
                    nc.gpsimd.memset(ctx_t[cc][:, :, 0:1], 0.0)
                    nc.gpsimd.memset(ctx_t[cc][:, :, W + 1 : W + 2], 0.0)

                # ========== attention blocks (fine-grained SW pipeline) ==========
                # Per block: front = qk/vT projections + e'=k^T q chains (ACT-paced
                # exp). back = denominator + av matmul chains (PE-dense). The PE
                # hardware queue is in-order, so back(b-1) chains are emitted
                # BETWEEN front(b) e' chains to keep the PE dense while ACT drains.
                def emit_qk_vt(bki):
                    q_sb = blk.tile([D, L], BF16, tag="q", name="q")
                    k_sb = blk.tile([D, L], BF16, tag="k", name="k")
                    for dst, w_sb, b_sb in (
                        (q_sb, wq_sb, bq_sb),
                        (k_sb, wk_sb, bk_sb),
                    ):
                        ps = pbig.tile([D, L], F32, tag="pbig", name="pbig")
                        for h in range(2):
                            for cc in range(CC):
                                nc.tensor.matmul(
                                    ps[:, h * 512 : (h + 1) * 512],
                                    w_sb[:, cc, :],
                                    xb_sb[cc][:, bki, h * 512 : (h + 1) * 512],
                                    start=(cc == 0),
                                    stop=(cc == CC - 1),
                                )
                        nc.vector.tensor_scalar_add(out=dst, in0=ps, scalar1=b_sb)
                    vt = blk.tile([128, 8, C], BF16, tag="vt", name="vt")
                    for mp in range(4):
                        ps = pmisc.tile([128, 512], F32, tag="pmisc", name="pmisc")
                        for half in range(2):
                            m = 2 * mp + half
                            for cc in range(CC):
                                nc.tensor.matmul(
                                    ps[:, half * C : (half + 1) * C],
                                    xb_sb[cc][:, bki, m * 128 : (m + 1) * 128],
                                    wv_sb[:, cc, :],
                                    start=(cc == 0),
                                    stop=(cc == CC - 1),
                                )
                        nc.vector.tensor_copy(
                            out=vt[:, 2 * mp : 2 * mp + 2, :],
                            in_=ps.rearrange("p (a c) -> p a c", a=2),
                        )
                    expE = blk.tile([128, 8, L], BF16, tag="expE", name="expE")
                    return dict(bki=bki, q=q_sb, k=k_sb, vt=vt, expE=expE)

                def emit_eprime(st, jc):
                    ps = pbig.tile([128, L], F32, tag="pbig", name="pbig")
                    for h in range(2):
                        nc.tensor.matmul(
                            ps[:, h * 512 : (h + 1) * 512],
                            st["k"][:, jc * 128 : (jc + 1) * 128],
                            st["q"][:, h * 512 : (h + 1) * 512],
                            start=True,
                            stop=True,
                        )
                    nc.scalar.activation(out=st["expE"][:, jc, :], in_=ps, func=AF.Exp)

                def back_steps(st):
                    """Generator yielding PE-dense chunks of the back half."""
                    bki, vt, expE = st["bki"], st["vt"], st["expE"]
                    si, sj = divmod(bki, S)
                    r0, c0 = si * HB, sj * HB
                    psd = pbig.tile([128, L], F32, tag="pbig", name="pbig")
                    for h in range(2):
                        for jc in range(8):
                            nc.tensor.matmul(
                                psd[:, h * 512 : (h + 1) * 512],
                                ones_sb,
                                expE[:, jc, h * 512 : (h + 1) * 512],
                                start=(jc == 0),
                                stop=(jc == 7),
                            )
                    rden = blk.tile([128, L], BF16, tag="rden", name="rden")
                    with nc.allow_low_precision("softmax weights tolerate bf16"):
                        nc.vector.reciprocal(out=rden, in_=psd)
                    yield
                    for cc in range(CC):
                        psa = pbig.tile([128, L], F32, tag="pbig", name="pbig")
                        for h in range(2):
                            for jc in range(8):
                                nc.tensor.matmul(
                                    psa[:, h * 512 : (h + 1) * 512],
                                    vt[:, jc, cc * 128 : (cc + 1) * 128],
                                    expE[:, jc, h * 512 : (h + 1) * 512],
                                    start=(jc == 0),
                                    stop=(jc == 7),
                                )
                            if h == 0:
                                yield
                        t_sb = blk.tile([128, L], BF16, tag="tsb", name="tsb")
                        nc.vector.tensor_mul(out=t_sb, in0=psa, in1=rden)
                        nc.vector.scalar_tensor_tensor(
                            out=ctx_t[cc][
                                :, 1 + r0 : 1 + r0 + HB, 1 + c0 : 1 + c0 + HB
                            ],
                            in0=t_sb.rearrange("p (h w) -> p h w", h=HB),
                            scalar=gbv_sb[:, cc, :],
                            in1=xb_sb[cc][:, bki, :].rearrange(
                                "p (h w) -> p h w", h=HB
                            ),
                            op0=ALU.add,
                            op1=ALU.add,
                        )
                        yield

                def drain(gen):
                    if gen is not None:
                        for _ in gen:
                            pass

                pending = None
                for bki in range(S * S):
                    st = emit_qk_vt(bki)
                    back = back_steps(pending) if pending is not None else None
                    # interleave: e' chains alternate with PE-dense back chunks
                    for jc in range(8):
                        emit_eprime(st, jc)
                        if back is not None and jc in (1, 3, 5, 6, 7):
                            next(back, None)
                    drain(back)
                    pending = st
                drain(back_steps(pending))

                # ================= conv 3x3 + BN + relu =================
                for oc in range(CC):
                    for t8 in range(8):  # 8 row-groups of 8 rows (N=512)
                        ps = pmisc.tile([128, 512], F32, tag="pmisc", name="pmisc")
                        first = True
                        for cc in range(CC):
                            for dy in range(3):
                                for dx in range(3):
                                    sh = dy * 3 + dx
                                    nc.tensor.matmul(
                                        ps,
                                        cw_sb[:, cc, sh, oc * 128 : (oc + 1) * 128],
                                        ctx_t[cc][
                                            :, t8 * 8 + dy : t8 * 8 + dy + 8, dx : dx + W
                                        ],
                                        start=first,
                                        stop=(cc == CC - 1 and sh == 8),
                                    )
                                    first = False
                        xpre_sb = cvp.tile([128, 512], F32, tag="xpre", name="xpre")
                        nc.sync.dma_start(
                            out=xpre_sb,
                            in_=xpre_h.ap()[
                                ex, oc * 128 : (oc + 1) * 128, t8 * 512 : (t8 + 1) * 512
                            ],
                        )
                        y_sb = cvp.tile([128, 512], F32, tag="ysb", name="ysb")
                        nc.vector.scalar_tensor_tensor(
                            out=y_sb,
                            in0=ps,
                            scalar=as_sb[:, oc, :],
                            in1=xpre_sb,
                            op0=ALU.mult,
                            op1=ALU.add,
                        )
                        nc.vector.tensor_scalar_max(out=y_sb, in0=y_sb, scalar1=0.0)
                        nc.sync.dma_start(
                            out=out_h.ap()[
                                ex, oc * 128 : (oc + 1) * 128, t8 * 512 : (t8 + 1) * 512
                            ],
                            in_=y_sb,
                        )

    nc.finalize()
    return nc


def _prep(inputs):
    """Host-side prep: fold constants, convert dtypes, build per-core in_maps."""
    bf = ml_dtypes.bfloat16
    x = np.ascontiguousarray(inputs["x"], dtype=np.float32)
    Wq, bq = inputs["Wq"], inputs["bq"]
    Wk, bk = inputs["Wk"], inputs["bk"]
    Wv, bv = inputs["Wv"], inputs["bv"]
    gnl = float(np.asarray(inputs["gamma_nl"]).reshape(-1)[0])
    gamma = float(np.asarray(inputs["gamma"]).reshape(-1)[0])
    convW, convb = inputs["convW"], inputs["convb"]
    bn_w, bn_b = inputs["bn_w"], inputs["bn_b"]
    bn_mean, bn_var = inputs["bn_mean"], inputs["bn_var"]

    inv = np.asarray(bn_w) / np.sqrt(np.asarray(bn_var) + BN_EPS)
    As = (gamma * inv).astype(np.float32).reshape(CC, 128, 1)
    Bs = (gamma * ((np.asarray(convb) - np.asarray(bn_mean)) * inv + np.asarray(bn_b))).astype(
        np.float32
    )

    x_flat = x.reshape(B, C, H * W)
    # block-major spatial order for the attention path
    x16 = np.ascontiguousarray(
        x.reshape(B, C, S, HB, S, HB).transpose(0, 1, 2, 4, 3, 5).reshape(B, C, H * W)
    ).astype(bf)
    xpre = x_flat + Bs[None, :, None]

    ginv = 1.0 / gnl if abs(gnl) > 1e-30 else 1.0
    shared = {
        "wqT": np.ascontiguousarray(np.asarray(Wq).T.reshape(CC, 128, D)).astype(bf),
        "wkT": np.ascontiguousarray(np.asarray(Wk).T.reshape(CC, 128, D)).astype(bf),
        "bq": np.asarray(bq, np.float32).reshape(D, 1),
        "bk": np.asarray(bk, np.float32).reshape(D, 1),
        "wvT": np.ascontiguousarray(np.asarray(Wv).T.reshape(CC, 128, C)).astype(bf),
        "onesg": np.full((128, 128), ginv, np.float32).astype(bf),
        "gnlbv": (gnl * np.asarray(bv, np.float32)).reshape(CC, 128, 1),
        # convW [O, I, 3, 3] -> per shift lhsT [c_in, o]: [ky,kx,ci,o] -> [cc,128,9,O]
        "cw": np.ascontiguousarray(
            np.asarray(convW).transpose(2, 3, 1, 0).reshape(9, CC, 128, C).transpose(1, 2, 0, 3)
        ).astype(bf),
        "As": As,
    }
    in_maps = []
    for core in range(N_CORES):
        m = dict(shared)
        m["x16"] = np.ascontiguousarray(x16[core * BL : (core + 1) * BL])
        m["xpre"] = np.ascontiguousarray(xpre[core * BL : (core + 1) * BL])
        in_maps.append(m)
    return in_maps


def kernel(**inputs) -> np.ndarray:
    if "nc" not in _cache:
        _cache["nc"] = build_bass()
    nc = _cache["nc"]
    in_maps = _prep(inputs)
    res = run_bass_kernel_spmd(nc, in_maps, core_ids=list(range(N_CORES)))
    out = np.concatenate([res.results[i]["out"] for i in range(N_CORES)], axis=0)
    return out.reshape(B, C, H, W).astype(np.float32)


if __name__ == "__main__":
    print("building...")
    build_bass()
    print("built ok")
